# revision 34
# baseline (speedup 1.0000x reference)
"""Trainium2 Bass kernel for nn_DensePoseV1ConvXGNInsHead:
2x (conv3x3 64->64 -> per-instance BN -> ReLU) on [8,64,256,256],
data-parallel one image per NeuronCore across 8 cores.

Self-contained: only imports the system concourse stack from /opt/trn_rl_repo.
"""
import os
import sys
import types

sys.path.insert(0, "/opt/trn_rl_repo")

import numpy as np

import concourse.bass as bass
import concourse.tile as tile
from concourse import mybir
from concourse.vector_clock import ScopedClock

f16 = mybir.dt.float16
f32 = mybir.dt.float32
ALU = mybir.AluOpType

C = 64          # channels
W = 256         # image width
PITCH = 272     # padded row pitch (16 left pad + 256 data; borrows next row's pad)
LP = 16         # left pad elements
R = 4           # conv rows per block (per half)
EPS = 1e-5

# ---------------------------------------------------------------------------
# walrus workaround: split the Tile exit-drain's sem waits (installed walrus
# rejects instructions with >2 sync waits)
# ---------------------------------------------------------------------------
_patched = False


def _install_tile_patch():
    global _patched
    if _patched:
        return
    _patched = True

    def _drain_and_barrier(self, tick_clock, wait_clock):
        nc = self.nc
        drain_inst = nc.sync.drain()
        wait_clock.add_sem_waits(
            drain_inst.ins, ScopedClock({None: tick_clock.global_clock})
        )
        si = drain_inst.ins.sync_info
        waits = list(si.on_wait or [])
        if len(waits) > 1:
            si.on_wait = waits[:1]
            for i in range(1, len(waits)):
                nop = nc.sync.nop()
                nop.ins.sync_info = mybir.SyncInfo(
                    on_wait=waits[i : i + 1], on_update=[]
                )
        nc.all_engine_barrier()
        popped = nc._tile_sem_poison_stack.pop()
        assert popped is self._sem_poison
        nc.clear_and_free_semaphores(list(self.sems.allocated().values()))
        nc.all_engine_barrier()

    tile.TileContext._drain_and_barrier = _drain_and_barrier


# ---------------------------------------------------------------------------
# NTFF profiling shim (antenv.axon_hooks is absent in this image)
# ---------------------------------------------------------------------------
def _install_ntff_shim():
    if "antenv.axon_hooks" in sys.modules:
        return
    mod = types.ModuleType("antenv.axon_hooks")
    state = {"hook": None}
    mod.set_axon_ntff_profile_hook = lambda h: state.__setitem__("hook", h)
    mod.get_axon_ntff_profile_hook = lambda: state["hook"]
    sys.modules["antenv.axon_hooks"] = mod
    try:
        import antenv

        antenv.axon_hooks = mod
    except ImportError:
        pass
    try:
        from trn_agent_boot.trn_boot import _ntff_profile_via_ctypes

        h = _ntff_profile_via_ctypes("/opt/axon/libaxon_pjrt.so")
        mod.set_axon_ntff_profile_hook(h)
    except Exception:
        pass


def yoff(slot):
    return slot * PITCH + LP


def _ap(base_ap, offset_elems, dims):
    """Build a sub-AP of base_ap at +offset (elements), with given free dims."""
    return bass.AP(
        tensor=base_ap.tensor,
        offset=base_ap.offset + offset_elems,
        ap=[base_ap.ap[0]] + dims,
    )


def emit(nc, H):
    """Emit the full 2-layer kernel for an HxW image (H=256 in production)."""
    HH = H // 2
    NB = HH // R            # conv blocks per layer
    NST = HH * 2            # stats chunks (128 px each) per layer
    NG = HH // 2            # normalize chunks (2 rows x both halves) per layer
    HW2 = HH * W
    assert HH % R == 0

    xh = nc.declare_dram_parameter("xh", [C, H * W], f16, isOutput=False)
    maskd = nc.declare_dram_parameter("maskd", [128, NST * 18], f16, isOutput=False)
    ms2ad = nc.declare_dram_parameter("ms2ad", [9, HW2], f16, isOutput=False)
    ms2bd = nc.declare_dram_parameter("ms2bd", [9, HW2], f16, isOutput=False)
    rcnt = nc.declare_dram_parameter("rcnt", [9], f32, isOutput=False)
    w0d = nc.declare_dram_parameter("w0d", [128, 9, 128], f16, isOutput=False)
    w1d = nc.declare_dram_parameter("w1d", [128, 9, 128], f16, isOutput=False)
    id128 = nc.declare_dram_parameter("id128", [128, 128], f16, isOutput=False)
    id18f = nc.declare_dram_parameter("id18f", [18, 18], f32, isOutput=False)
    g0 = nc.declare_dram_parameter("g0", [C], f32, isOutput=False)
    b0 = nc.declare_dram_parameter("b0", [C], f32, isOutput=False)
    g1 = nc.declare_dram_parameter("g1", [C], f32, isOutput=False)
    b1 = nc.declare_dram_parameter("b1", [C], f32, isOutput=False)
    out = nc.declare_dram_parameter("out", [C, H * W], f16, isOutput=True)

    with tile.TileContext(nc) as tc:
        import contextlib

        with contextlib.ExitStack() as ctx:
            const = ctx.enter_context(tc.tile_pool(name="const", bufs=1))
            stripp = ctx.enter_context(tc.tile_pool(name="stripp", bufs=3))
            msp = ctx.enter_context(tc.tile_pool(name="msp", bufs=3))
            nrm = ctx.enter_context(tc.tile_pool(name="nrm", bufs=6))
            stgp = ctx.enter_context(tc.tile_pool(name="stgp", bufs=3))
            smallp = ctx.enter_context(tc.tile_pool(name="smallp", bufs=2))
            ps = ctx.enter_context(tc.tile_pool(name="ps", bufs=4, space="PSUM"))
            pst = ctx.enter_context(tc.tile_pool(name="pst", bufs=1, space="PSUM"))
            pss = ctx.enter_context(tc.tile_pool(name="pss", bufs=1, space="PSUM"))

            # ---- layer-0 weights first on the sync queue so conv starts ASAP
            # (layer-1 weights are loaded at the top of layer 1)
            wts = []
            for wd in (w0d, w1d):
                wt = const.tile([128, 9, 128], f16, tag="wt")
                wts.append(wt)
            nc.sync.dma_start(out=wts[0][:, 0:3, :], in_=w0d[:, 0:3, :])
            nc.sync.dma_start(out=wts[0][:, 3:9, :], in_=w0d[:, 3:9, :])
            wz = const.tile([128, 64], f16)
            nc.vector.memset(wz[:], 0.0)
            pwarm = ps.tile([128, 512], f32, tag="c512", bufs=3, name="pwarm")
            for _ in range(45):
                nc.tensor.matmul(pwarm[0:64, 0:64], wz[0:64, :], wz[0:64, :],
                                 start=True, stop=True, tile_position=(0, 0))

            # ---- persistent y buffer + x staging (pads zeroed once)
            ysb = const.tile([128, (HH + 2) * PITCH + LP], f16)
            xb0 = const.tile([128, (R + 2) * PITCH + LP], f16, tag="xb0")
            xb1 = const.tile([128, (R + 2) * PITCH + LP], f16, tag="xb1")
            nc.vector.memset(xb0[:], 0.0)
            nc.vector.memset(xb1[:], 0.0)
            xbs = [xb0, xb1]
            # ysb: zero the pad strips + the two halo slots (0 and HH+1)
            nc.vector.memset(_ap(ysb[:], 0, [[PITCH, HH + 2], [1, LP]]), 0.0)
            nc.vector.memset(_ap(ysb[:], (HH + 2) * PITCH, [[1, LP]]), 0.0)
            nc.vector.memset(_ap(ysb[:], yoff(0), [[1, W]]), 0.0)
            nc.vector.memset(_ap(ysb[:], yoff(HH + 1), [[1, W]]), 0.0)

            # ---- constants on the scalar hwdge queue (off the critical path)
            id128sb = const.tile([128, 128], f16)
            nc.scalar.dma_start(out=id128sb[:], in_=id128[:])
            id18sb = const.tile([18, 18], f32)
            nc.scalar.dma_start(out=id18sb[:], in_=id18f[:])
            maskpm = const.tile([128, NST * 18], f16)
            nc.scalar.dma_start(out=maskpm[:], in_=maskd[:])
            rcsb = const.tile([9, 1], f32)
            nc.scalar.dma_start(out=rcsb[:], in_=rcnt[:].rearrange("(a b) -> a b", b=1))
            epsap = const.tile([9, 1], f32)
            nc.vector.memset(epsap[:], EPS)
            one16 = const.tile([1, 64], f16)
            zero16 = const.tile([1, 64], f16)
            nc.vector.memset(one16[:], 1.0)
            nc.vector.memset(zero16[:], 0.0)
            gam = []
            bet = []
            for gg, bb in ((g0, b0), (g1, b1)):
                gt = const.tile([9, 64], f32, tag="gam")
                bt = const.tile([9, 64], f32, tag="bet")
                nc.scalar.dma_start(out=gt[:], in_=gg[:].partition_broadcast(9))
                nc.scalar.dma_start(out=bt[:], in_=bb[:].partition_broadcast(9))
                gam.append(gt)
                bet.append(bt)

            MCH = 4096          # ms2 stream chunk (8 normalize windows)
            NMG = HW2 // MCH    # ms2 groups per layer
            norm_emitters = [None, None]

            for L in (0, 1):
                wt = wts[L]
                if L == 1:
                    nc.scalar.dma_start(out=wts[1][:], in_=w1d[:])
                slot0 = 1 if L == 0 else 0   # y row r lives at slot r+slot0
                stats = pss.tile([18, 256], f32, tag="stats", name=f"stats{L}")
                # af/bf allocated early: background row (8) is constant and is
                # filled off the critical path; rows 0:8 come from the finalize
                af = smallp.tile([9, 64], f16, tag="af", name=f"af{L}")
                cf = smallp.tile([9, 64], f16, tag="cf", name=f"cf{L}")
                nc.scalar.dma_start(out=af[8:9, :], in_=one16[:])
                nc.scalar.dma_start(out=cf[8:9, :], in_=zero16[:])

                # ================= conv + stats (depth-2 pipeline) ==========
                pend = {}     # b -> psum chunks awaiting transpose/stats

                def do_conv(b):
                    r0 = b * R
                    if L == 0:
                        xb = xbs[b % 2]
                        if b == NB - 1:
                            # bottom halo of B half must be zero (slot R+1)
                            nc.vector.memset(
                                xb[64:128, (R + 1) * PITCH + LP : (R + 1) * PITCH + LP + W],
                                0.0,
                            )
                        lo_a = r0 - 1
                        s_a = 0
                        if b == 0:
                            lo_a, s_a = 0, 1
                        n_a = r0 + R - lo_a + 1
                        nc.sync.dma_start(
                            out=_ap(xb[0:64, :], yoff(s_a), [[PITCH, n_a], [1, W]]),
                            in_=bass.AP(
                                tensor=xh[:].tensor,
                                offset=lo_a * W,
                                ap=[[H * W, 64], [W, n_a], [1, W]],
                            ),
                        )
                        hb_lo = HH + r0 - 1
                        n_b = R + 2 if b < NB - 1 else R + 1
                        nc.sync.dma_start(
                            out=_ap(xb[64:128, :], yoff(0), [[PITCH, n_b], [1, W]]),
                            in_=bass.AP(
                                tensor=xh[:].tensor,
                                offset=hb_lo * W,
                                ap=[[H * W, 64], [W, n_b], [1, W]],
                            ),
                        )
                        src_t = xb
                        loc = lambda rr, dy: (rr - r0 + 1 + dy)  # slot in xb
                    else:
                        src_t = ysb
                        loc = lambda rr, dy: (rr + dy + 1)       # y1 slot

                    pts = [
                        ps.tile([128, 512], f32, tag="c512", bufs=3,
                                name=f"cps_{L}_{b}_{i}")
                        for i in range(R // 2)
                    ]
                    for t in range(9):
                        dy, dx = t // 3 - 1, t % 3 - 1
                        for half in (0, 1):
                            for cp in range(R // 2):
                                rr = r0 + 2 * cp
                                off = yoff(loc(rr, dy)) + dx
                                if half == 0:
                                    rhs = _ap(src_t[0:64, :], off, [[PITCH, 2], [1, W]])
                                    nc.tensor.matmul(
                                        pts[cp][0:64, :], wt[0:64, t, 0:64], rhs,
                                        start=(t == 0), stop=(t == 8),
                                        tile_position=(0, 0),
                                    )
                                else:
                                    rhs = _ap(src_t[64:128, :], off, [[PITCH, 2], [1, W]])
                                    nc.tensor.matmul(
                                        pts[cp][64:128, :], wt[64:128, t, 64:128], rhs,
                                        start=(t == 0), stop=(t == 8),
                                        tile_position=(64, 64),
                                    )
                    for cp in range(R // 2):
                        rr = r0 + 2 * cp
                        dst = _ap(ysb[:], yoff(rr + slot0), [[PITCH, 2], [1, W]])
                        nc.scalar.copy(out=dst, in_=pts[cp][:])

                def do_transpose(b):
                    r0 = b * R
                    pts2 = pst.tile([128, 1024], f16, tag="tp", name=f"tp_{L}_{b}")
                    for j in range(2 * R):
                        rr = r0 + j // 2
                        cs = j % 2
                        src = _ap(ysb[:], yoff(rr + slot0) + cs * 128, [[1, 128]])
                        nc.tensor.transpose(
                            pts2[:, j * 128 : (j + 1) * 128], src, id128sb[:]
                        )
                    sp = stripp.tile([128, 2 * R, 256], f16, tag="strip",
                                     name=f"sp_{L}_{b}")
                    nc.scalar.copy(
                        out=_ap(sp[:], 0, [[256, 2 * R], [1, 128]]),
                        in_=pts2[:],
                    )
                    nc.vector.tensor_tensor(
                        _ap(sp[:], 128, [[256, 2 * R], [1, 128]]),
                        _ap(sp[:], 0, [[256, 2 * R], [1, 128]]),
                        pts2[:],
                        ALU.mult,
                    )
                    pend[b] = sp

                def do_stats(b):
                    sp = pend.pop(b)
                    for j in range(2 * R):
                        ci = b * 2 * R + j
                        nc.tensor.matmul(
                            stats[:],
                            _ap(maskpm[:], ci * 18, [[1, 18]]),
                            sp[:, j, :],
                            start=(ci == 0), stop=(ci == NST - 1),
                        )

                for b in range(NB):
                    if L == 1 and b >= 1:
                        # weave the previous layer's normalize trail between
                        # this layer's conv blocks: keep ~2 chunks ahead of
                        # what block b's taps read (rows <= 4b+4)
                        for g in (2 * b + 3, 2 * b + 4):
                            if g <= NG - 2:
                                norm_emitters[0](g)
                    do_conv(b)
                    if b >= 1:
                        do_transpose(b - 1)
                    if b >= 2:
                        do_stats(b - 2)
                do_transpose(NB - 1)
                do_stats(NB - 2)
                do_stats(NB - 1)

                # ================= stats finalize =================
                ssb = smallp.tile([18, 256], f32, tag="ssb")
                nc.scalar.copy(out=ssb[:], in_=stats[:])
                fold = ps.tile([128, 512], f32, tag="exp", bufs=3,
                               name=f"fold{L}")
                nc.tensor.matmul(fold[0:9, 0:128], id18sb[:, 0:9],
                                 _ap(ssb[:], 0, [[128, 2], [1, 64]]),
                                 start=True, stop=False, skip_group_check=True)
                nc.tensor.matmul(fold[0:9, 0:128], id18sb[:, 9:18],
                                 _ap(ssb[:], 64, [[128, 2], [1, 64]]),
                                 start=False, stop=True, skip_group_check=True)
                s12 = smallp.tile([9, 128], f32, tag="s12")
                nc.scalar.copy(out=s12[:], in_=fold[0:9, 0:128])
                mean = smallp.tile([9, 64], f32, tag="mean")
                nc.vector.tensor_scalar_mul(out=mean[:], in0=s12[:, 0:64],
                                            scalar1=rcsb[:])
                e2 = smallp.tile([9, 64], f32, tag="e2")
                nc.vector.tensor_scalar_mul(out=e2[:], in0=s12[:, 64:128],
                                            scalar1=rcsb[:])
                var = smallp.tile([9, 64], f32, tag="var")
                nc.vector.tensor_tensor(var[:], mean[:], mean[:], ALU.mult)
                nc.vector.tensor_tensor(var[:], e2[:], var[:], ALU.subtract)
                sd = smallp.tile([9, 64], f32, tag="sd")
                nc.scalar.activation(
                    out=sd[:], in_=var[:], func=mybir.ActivationFunctionType.Sqrt,
                    bias=epsap[:], scale=1.0,
                )
                rstd = smallp.tile([9, 64], f32, tag="rstd")
                nc.vector.reciprocal(out=rstd[:], in_=sd[:])
                af32 = smallp.tile([9, 64], f32, tag="af32")
                nc.vector.tensor_tensor(af32[:], rstd[:], gam[L][:], ALU.mult)
                nc.vector.tensor_copy(af[0:8, :], af32[0:8, :])
                mA = smallp.tile([9, 64], f32, tag="mA")
                nc.vector.tensor_tensor(mA[:], mean[:], af32[:], ALU.mult)
                bf32 = smallp.tile([9, 64], f32, tag="bf32")
                nc.vector.tensor_tensor(bf32[:], bet[L][:], mA[:], ALU.subtract)
                nc.vector.tensor_copy(cf[0:8, :], bf32[0:8, :])

                # ================= normalize =================
                def load_ms(mg, sfx):
                    msa = msp.tile([9, MCH], f16, tag="msa", name=f"msa{L}_{sfx}")
                    msb = msp.tile([9, MCH], f16, tag="msb", name=f"msb{L}_{sfx}")
                    nc.sync.dma_start(
                        out=msa[:],
                        in_=bass.AP(tensor=ms2ad[:].tensor, offset=mg * MCH,
                                    ap=[[HW2, 9], [1, MCH]]),
                    )
                    nc.sync.dma_start(
                        out=msb[:],
                        in_=bass.AP(tensor=ms2bd[:].tensor, offset=mg * MCH,
                                    ap=[[HW2, 9], [1, MCH]]),
                    )
                    return msa, msb

                # L0 emits the LAST chunk first so both inter-layer halo rows
                # exist early; then a few leading chunks pre-seed, and the
                # remaining chunks are woven between the next layer's conv
                # blocks (see the b-loop above).
                if L == 0:
                    gmap = {NMG - 1: load_ms(NMG - 1, "p7"),
                            0: load_ms(0, "p0"), 1: load_ms(1, "p1")}
                else:
                    gmap = {0: load_ms(0, "p0"), 1: load_ms(1, "p1")}
                stg_box = [None]

                def emit_norm(g, L=L, slot0=slot0, af=af, cf=cf, gmap=gmap,
                              load_ms=load_ms, stg_box=stg_box):
                    if g % 8 == 6:
                        mg2 = g // 8 + 2
                        if (L == 0 and g <= NG - 18) or (L == 1 and mg2 < NMG):
                            gmap[mg2] = load_ms(mg2, f"l{g}")
                    mg = g // 8
                    msa, msb = gmap[mg]
                    j = g % 8
                    base = yoff(2 * g + slot0)
                    # layer 1's normalize runs after all conv work: the conv
                    # psum banks are idle, so alternate tags for 6-bank depth
                    if L == 1:
                        etag, ebufs = ("c512", 3) if g % 2 else ("exp", 3)
                    else:
                        etag, ebufs = "exp", 3
                    sE = ps.tile([128, 512], f32, tag=etag, bufs=ebufs,
                                 name=f"se{L}_{g}")
                    oE = ps.tile([128, 512], f32, tag=etag, bufs=ebufs,
                                 name=f"oe{L}_{g}")
                    winA = msa[:, j * 512 : (j + 1) * 512]
                    winB = msb[:, j * 512 : (j + 1) * 512]
                    yv = _ap(ysb[:], base, [[PITCH, 2], [1, W]])
                    nc.tensor.matmul(sE[0:64, :], af[:], winA,
                                     start=True, stop=True, tile_position=(0, 0))
                    nc.tensor.matmul(sE[64:128, :], af[:], winB,
                                     start=True, stop=True, tile_position=(0, 64))
                    nc.tensor.matmul(oE[0:64, :], cf[:], winA,
                                     start=True, stop=True, tile_position=(0, 0))
                    nc.tensor.matmul(oE[64:128, :], cf[:], winB,
                                     start=True, stop=True, tile_position=(0, 64))
                    sEc = nrm.tile([128, 512], f16, tag="sEc", name=f"sEc{L}_{g}")
                    oEc = nrm.tile([128, 512], f16, tag="oEc", name=f"oEc{L}_{g}")
                    nc.scalar.copy(out=sEc[:], in_=sE[:])
                    nc.scalar.copy(out=oEc[:], in_=oE[:])
                    t1 = nrm.tile([128, 512], f16, tag="t1", name=f"t1{L}_{g}")
                    t2 = nrm.tile([128, 512], f16, tag="t2", name=f"t2{L}_{g}")
                    nc.vector.tensor_tensor(t1[:], yv, sEc[:], ALU.mult)
                    nc.vector.tensor_tensor(t2[:], t1[:], oEc[:], ALU.add)
                    if L == 0:
                        dst_relu = yv
                    else:
                        if g % 4 == 0:
                            stg_box[0] = stgp.tile([128, 2048], f16, tag="stg",
                                                   name=f"stg{g // 4}")
                        stg = stg_box[0]
                        dst_relu = stg[:, (g % 4) * 512 : (g % 4 + 1) * 512]
                    nc.vector.tensor_scalar_max(out=dst_relu, in0=t2[:],
                                                scalar1=0.0)
                    if L == 0:
                        if g == 0:
                            # halo: A slot HH+1 <- B row 0 (normalized)
                            nc.sync.dma_start(
                                out=_ap(ysb[0:64, :], yoff(HH + 1), [[1, W]]),
                                in_=_ap(ysb[64:128, :], yoff(1), [[1, W]]),
                            )
                        if g == NG - 1:
                            # halo: B slot 0 <- A row HH-1 (normalized)
                            nc.sync.dma_start(
                                out=_ap(ysb[64:128, :], yoff(0), [[1, W]]),
                                in_=_ap(ysb[0:64, :], yoff(HH), [[1, W]]),
                            )
                    else:
                        if g % 4 == 3:
                            grp = g // 4
                            nc.sync.dma_start(
                                out=bass.AP(
                                    tensor=out[:].tensor,
                                    offset=grp * 8 * W,
                                    ap=[[H * W, 64], [1, 2048]],
                                ),
                                in_=stg[0:64, :],
                            )
                            nc.sync.dma_start(
                                out=bass.AP(
                                    tensor=out[:].tensor,
                                    offset=HH * W + grp * 8 * W,
                                    ap=[[H * W, 64], [1, 2048]],
                                ),
                                in_=stg[64:128, :],
                            )

                norm_emitters[L] = emit_norm
                if L == 0:
                    # last chunk first (frees the B halo), then pre-seed the
                    # first few rows; the rest is woven into layer 1's conv
                    for g in (NG - 1, 0, 1, 2, 3, 4):
                        emit_norm(g)
                else:
                    for g in range(NG):
                        emit_norm(g)

    return nc


MAXW = 1


def _split_multi_waits(nc):
    """The installed walrus rejects instructions with >MAXW sync waits; hoist
    excess waits onto preceding same-engine nops."""
    nsplit = 0
    for fn in nc.m.functions:
        for blk in fn.blocks:
            insts = list(blk.instructions)
            out = []
            for inst in insts:
                si = inst.sync_info
                waits = list(si.on_wait) if (si and si.on_wait) else []
                if len(waits) > MAXW:
                    for i in range(0, len(waits) - MAXW, MAXW):
                        nop = mybir.InstNoOp(
                            name=f"WSPLIT-{nsplit}", ins=[], outs=[]
                        )
                        nsplit += 1
                        nop.engine = inst.engine
                        nop.sync_info = mybir.SyncInfo(
                            on_wait=waits[i : i + MAXW], on_update=[]
                        )
                        out.append(nop)
                    si.on_wait = waits[len(waits) - MAXW :]
                out.append(inst)
            if len(out) != len(insts):
                while len(blk.instructions):
                    blk.instructions.pop()
                for inst in out:
                    blk.instructions.append(inst)
    return nsplit


def build_nc(H=256, split_waits=True):
    _install_tile_patch()
    nc = bass.Bass()
    emit(nc, H)
    if split_waits:
        n = _split_multi_waits(nc)
        if n:
            print(f"kernel: split {n} multi-wait instructions")
    return nc


# ---------------------------------------------------------------------------
# host-side input prep
# ---------------------------------------------------------------------------
def prep_core_inputs(x_img, ids_img, w0, g0v, b0v, w1, g1v, b1v, H=256):
    """x_img [C,H,W] f32, ids_img [H,W] int -> input map for one core."""
    HH = H // 2
    NST = HH * 2
    seg = np.where(ids_img < 0, 8, ids_img).astype(np.int64)

    m = {}
    m["xh"] = np.ascontiguousarray(x_img.reshape(C, H * W).astype(np.float16))
    cnt = np.bincount(seg.reshape(-1), minlength=9)[:9]
    m["rcnt"] = (1.0 / np.maximum(cnt, 1)).astype(np.float32)

    # one-hot masks, host-precomputed
    # maskd[p, (2*rr+cs)*18 + 9*h + s] = (ids[h*HH+rr, cs*128+p] == vals[s])
    idh = ids_img.reshape(2, HH, 2, 128)              # [h, rr, cs, p]
    vals = np.arange(9)
    vals[8] = -1
    mk = (idh[..., None] == vals).astype(np.float16)  # [h, rr, cs, p, s]
    mk = mk.transpose(3, 1, 2, 0, 4)                  # [p, rr, cs, h, s]
    m["maskd"] = np.ascontiguousarray(mk.reshape(128, NST * 18))
    # segment-major masks per half: ms2{a,b}[s, px]
    flat = ids_img.reshape(2, HH * W)
    msk2 = (flat[:, None, :] == vals[None, :, None]).astype(np.float16)  # [2,9,px]
    m["ms2ad"] = np.ascontiguousarray(msk2[0])
    m["ms2bd"] = np.ascontiguousarray(msk2[1])

    for name, wmat in (("w0d", w0), ("w1d", w1)):
        wd = np.zeros((9, 128, 128), np.float16)
        for t in range(9):
            dy, dx = t // 3, t % 3
            lhsT = wmat[:, :, dy, dx].T.astype(np.float16)  # [cin, cout]
            wd[t, 0:64, 0:64] = lhsT
            wd[t, 64:128, 64:128] = lhsT
        m[name] = np.ascontiguousarray(wd.transpose(1, 0, 2))  # [ci, t, co]

    m["id128"] = np.eye(128, dtype=np.float16)
    m["id18f"] = np.eye(18, dtype=np.float32)
    m["g0"] = np.asarray(g0v, np.float32)
    m["b0"] = np.asarray(b0v, np.float32)
    m["g1"] = np.asarray(g1v, np.float32)
    m["b1"] = np.asarray(b1v, np.float32)
    return m


LAST_RESULT = None


def kernel(features, ins_indices_batch, w0, g0, b0, w1, g1, b1):
    global LAST_RESULT
    _install_ntff_shim()
    from concourse.bass_utils import run_bass_kernel_spmd
    from concourse import bass2jax as _b2j
    import traceback as _tb

    _b2j.install_neuronx_cc_hook()
    import libneuronxla as _lnx

    if not getattr(_lnx, "_ant_dbg_wrapped", False):
        _orig = _lnx.neuronx_cc

        def _dbg(*a, **k):
            try:
                return _orig(*a, **k)
            except BaseException:
                _tb.print_exc()
                raise

        _lnx.neuronx_cc = _dbg
        _lnx._ant_dbg_wrapped = True

    x = np.asarray(features, np.float32)
    ids = np.asarray(ins_indices_batch).astype(np.int64)
    w0 = np.asarray(w0, np.float32)
    w1 = np.asarray(w1, np.float32)
    N = x.shape[0]
    H = x.shape[2]

    nc = build_nc(H)
    in_maps = [
        prep_core_inputs(x[i], ids[i], w0, g0, b0, w1, g1, b1, H) for i in range(N)
    ]
    trace = bool(int(os.environ.get("BASS_KERNEL_TRACE", "0")))
    res = run_bass_kernel_spmd(nc, in_maps, list(range(N)), trace=trace)
    LAST_RESULT = res
    outs = [
        res.results[i]["out"].astype(np.float32).reshape(C, H, W) for i in range(N)
    ]
    return np.stack(outs, 0)


# revision 35
# speedup vs baseline: 1.0063x; 1.0063x over previous
"""Trainium2 Bass kernel for nn_DensePoseV1ConvXGNInsHead:
2x (conv3x3 64->64 -> per-instance BN -> ReLU) on [8,64,256,256],
data-parallel one image per NeuronCore across 8 cores.

Self-contained: only imports the system concourse stack from /opt/trn_rl_repo.
"""
import os
import sys
import types

sys.path.insert(0, "/opt/trn_rl_repo")

import numpy as np

import concourse.bass as bass
import concourse.tile as tile
from concourse import mybir
from concourse.vector_clock import ScopedClock

f16 = mybir.dt.float16
f32 = mybir.dt.float32
ALU = mybir.AluOpType

C = 64          # channels
W = 256         # image width
PITCH = 272     # padded row pitch (16 left pad + 256 data; borrows next row's pad)
LP = 16         # left pad elements
R = 4           # conv rows per block (per half)
EPS = 1e-5

# ---------------------------------------------------------------------------
# walrus workaround: split the Tile exit-drain's sem waits (installed walrus
# rejects instructions with >2 sync waits)
# ---------------------------------------------------------------------------
_patched = False


def _install_tile_patch():
    global _patched
    if _patched:
        return
    _patched = True

    def _drain_and_barrier(self, tick_clock, wait_clock):
        nc = self.nc
        drain_inst = nc.sync.drain()
        wait_clock.add_sem_waits(
            drain_inst.ins, ScopedClock({None: tick_clock.global_clock})
        )
        si = drain_inst.ins.sync_info
        waits = list(si.on_wait or [])
        if len(waits) > 1:
            si.on_wait = waits[:1]
            for i in range(1, len(waits)):
                nop = nc.sync.nop()
                nop.ins.sync_info = mybir.SyncInfo(
                    on_wait=waits[i : i + 1], on_update=[]
                )
        nc.all_engine_barrier()
        popped = nc._tile_sem_poison_stack.pop()
        assert popped is self._sem_poison
        nc.clear_and_free_semaphores(list(self.sems.allocated().values()))
        nc.all_engine_barrier()

    tile.TileContext._drain_and_barrier = _drain_and_barrier


# ---------------------------------------------------------------------------
# NTFF profiling shim (antenv.axon_hooks is absent in this image)
# ---------------------------------------------------------------------------
def _install_ntff_shim():
    if "antenv.axon_hooks" in sys.modules:
        return
    mod = types.ModuleType("antenv.axon_hooks")
    state = {"hook": None}
    mod.set_axon_ntff_profile_hook = lambda h: state.__setitem__("hook", h)
    mod.get_axon_ntff_profile_hook = lambda: state["hook"]
    sys.modules["antenv.axon_hooks"] = mod
    try:
        import antenv

        antenv.axon_hooks = mod
    except ImportError:
        pass
    try:
        from trn_agent_boot.trn_boot import _ntff_profile_via_ctypes

        h = _ntff_profile_via_ctypes("/opt/axon/libaxon_pjrt.so")
        mod.set_axon_ntff_profile_hook(h)
    except Exception:
        pass


def yoff(slot):
    return slot * PITCH + LP


def _ap(base_ap, offset_elems, dims):
    """Build a sub-AP of base_ap at +offset (elements), with given free dims."""
    return bass.AP(
        tensor=base_ap.tensor,
        offset=base_ap.offset + offset_elems,
        ap=[base_ap.ap[0]] + dims,
    )


def emit(nc, H):
    """Emit the full 2-layer kernel for an HxW image (H=256 in production)."""
    HH = H // 2
    NB = HH // R            # conv blocks per layer
    NST = HH * 2            # stats chunks (128 px each) per layer
    NG = HH // 2            # normalize chunks (2 rows x both halves) per layer
    HW2 = HH * W
    assert HH % R == 0

    xh = nc.declare_dram_parameter("xh", [C, H * W], f16, isOutput=False)
    maskd = nc.declare_dram_parameter("maskd", [128, NST * 18], f16, isOutput=False)
    ms2ad = nc.declare_dram_parameter("ms2ad", [9, HW2], f16, isOutput=False)
    ms2bd = nc.declare_dram_parameter("ms2bd", [9, HW2], f16, isOutput=False)
    rcnt = nc.declare_dram_parameter("rcnt", [9], f32, isOutput=False)
    w0d = nc.declare_dram_parameter("w0d", [128, 9, 128], f16, isOutput=False)
    w1d = nc.declare_dram_parameter("w1d", [128, 9, 128], f16, isOutput=False)
    id128 = nc.declare_dram_parameter("id128", [128, 128], f16, isOutput=False)
    id18f = nc.declare_dram_parameter("id18f", [18, 18], f32, isOutput=False)
    g0 = nc.declare_dram_parameter("g0", [C], f32, isOutput=False)
    b0 = nc.declare_dram_parameter("b0", [C], f32, isOutput=False)
    g1 = nc.declare_dram_parameter("g1", [C], f32, isOutput=False)
    b1 = nc.declare_dram_parameter("b1", [C], f32, isOutput=False)
    out = nc.declare_dram_parameter("out", [C, H * W], f16, isOutput=True)

    with tile.TileContext(nc) as tc:
        import contextlib

        with contextlib.ExitStack() as ctx:
            const = ctx.enter_context(tc.tile_pool(name="const", bufs=1))
            stripp = ctx.enter_context(tc.tile_pool(name="stripp", bufs=3))
            msp = ctx.enter_context(tc.tile_pool(name="msp", bufs=3))
            nrm = ctx.enter_context(tc.tile_pool(name="nrm", bufs=6))
            stgp = ctx.enter_context(tc.tile_pool(name="stgp", bufs=3))
            smallp = ctx.enter_context(tc.tile_pool(name="smallp", bufs=2))
            ps = ctx.enter_context(tc.tile_pool(name="ps", bufs=4, space="PSUM"))
            pst = ctx.enter_context(tc.tile_pool(name="pst", bufs=1, space="PSUM"))
            pss = ctx.enter_context(tc.tile_pool(name="pss", bufs=1, space="PSUM"))

            # ---- layer-0 weights first on the sync queue so conv starts ASAP
            # (layer-1 weights are loaded at the top of layer 1)
            wts = []
            for wd in (w0d, w1d):
                wt = const.tile([128, 9, 128], f16, tag="wt")
                wts.append(wt)
            nc.sync.dma_start(out=wts[0][:, 0:3, :], in_=w0d[:, 0:3, :])
            nc.sync.dma_start(out=wts[0][:, 3:9, :], in_=w0d[:, 3:9, :])
            wz = const.tile([128, 64], f16)
            nc.vector.memset(wz[:], 0.0)
            pwarm = ps.tile([128, 512], f32, tag="c512", bufs=3, name="pwarm")
            for _ in range(45):
                nc.tensor.matmul(pwarm[0:64, 0:64], wz[0:64, :], wz[0:64, :],
                                 start=True, stop=True, tile_position=(0, 0))

            # ---- persistent y buffer + x staging (pads zeroed once)
            ysb = const.tile([128, (HH + 2) * PITCH + LP], f16)
            xb0 = const.tile([128, (R + 2) * PITCH + LP], f16, tag="xb0")
            xb1 = const.tile([128, (R + 2) * PITCH + LP], f16, tag="xb1")
            nc.vector.memset(xb0[:], 0.0)
            nc.vector.memset(xb1[:], 0.0)
            xbs = [xb0, xb1]
            # ysb: zero the pad strips + the two halo slots (0 and HH+1)
            nc.vector.memset(_ap(ysb[:], 0, [[PITCH, HH + 2], [1, LP]]), 0.0)
            nc.vector.memset(_ap(ysb[:], (HH + 2) * PITCH, [[1, LP]]), 0.0)
            nc.vector.memset(_ap(ysb[:], yoff(0), [[1, W]]), 0.0)
            nc.vector.memset(_ap(ysb[:], yoff(HH + 1), [[1, W]]), 0.0)

            # ---- constants on the scalar hwdge queue (off the critical path)
            id128sb = const.tile([128, 128], f16)
            nc.scalar.dma_start(out=id128sb[:], in_=id128[:])
            id18sb = const.tile([18, 18], f32)
            nc.scalar.dma_start(out=id18sb[:], in_=id18f[:])
            maskpm = const.tile([128, NST * 18], f16)
            nc.scalar.dma_start(out=maskpm[:], in_=maskd[:])
            rcsb = const.tile([9, 1], f32)
            nc.scalar.dma_start(out=rcsb[:], in_=rcnt[:].rearrange("(a b) -> a b", b=1))
            epsap = const.tile([9, 1], f32)
            nc.vector.memset(epsap[:], EPS)
            one16 = const.tile([1, 64], f16)
            zero16 = const.tile([1, 64], f16)
            nc.vector.memset(one16[:], 1.0)
            nc.vector.memset(zero16[:], 0.0)
            gam = []
            bet = []
            for gg, bb in ((g0, b0), (g1, b1)):
                gt = const.tile([9, 64], f32, tag="gam")
                bt = const.tile([9, 64], f32, tag="bet")
                nc.scalar.dma_start(out=gt[:], in_=gg[:].partition_broadcast(9))
                nc.scalar.dma_start(out=bt[:], in_=bb[:].partition_broadcast(9))
                gam.append(gt)
                bet.append(bt)

            MCH = 4096          # ms2 stream chunk (8 normalize windows)
            NMG = HW2 // MCH    # ms2 groups per layer
            norm_emitters = [None, None]

            for L in (0, 1):
                wt = wts[L]
                if L == 1:
                    nc.scalar.dma_start(out=wts[1][:], in_=w1d[:])
                slot0 = 1 if L == 0 else 0   # y row r lives at slot r+slot0
                stats = pss.tile([18, 256], f32, tag="stats", name=f"stats{L}")
                # af/bf allocated early: background row (8) is constant and is
                # filled off the critical path; rows 0:8 come from the finalize
                af = smallp.tile([9, 64], f16, tag="af", name=f"af{L}")
                cf = smallp.tile([9, 64], f16, tag="cf", name=f"cf{L}")
                nc.scalar.dma_start(out=af[8:9, :], in_=one16[:])
                nc.scalar.dma_start(out=cf[8:9, :], in_=zero16[:])

                # ================= conv + stats (depth-2 pipeline) ==========
                pend = {}     # b -> psum chunks awaiting transpose/stats

                def do_conv(b):
                    r0 = b * R
                    if L == 0:
                        xb = xbs[b % 2]
                        if b == NB - 1:
                            # bottom halo of B half must be zero (slot R+1)
                            nc.vector.memset(
                                xb[64:128, (R + 1) * PITCH + LP : (R + 1) * PITCH + LP + W],
                                0.0,
                            )
                        lo_a = r0 - 1
                        s_a = 0
                        if b == 0:
                            lo_a, s_a = 0, 1
                        n_a = r0 + R - lo_a + 1
                        nc.sync.dma_start(
                            out=_ap(xb[0:64, :], yoff(s_a), [[PITCH, n_a], [1, W]]),
                            in_=bass.AP(
                                tensor=xh[:].tensor,
                                offset=lo_a * W,
                                ap=[[H * W, 64], [W, n_a], [1, W]],
                            ),
                        )
                        hb_lo = HH + r0 - 1
                        n_b = R + 2 if b < NB - 1 else R + 1
                        nc.sync.dma_start(
                            out=_ap(xb[64:128, :], yoff(0), [[PITCH, n_b], [1, W]]),
                            in_=bass.AP(
                                tensor=xh[:].tensor,
                                offset=hb_lo * W,
                                ap=[[H * W, 64], [W, n_b], [1, W]],
                            ),
                        )
                        src_t = xb
                        loc = lambda rr, dy: (rr - r0 + 1 + dy)  # slot in xb
                    else:
                        src_t = ysb
                        loc = lambda rr, dy: (rr + dy + 1)       # y1 slot

                    pts = [
                        ps.tile([128, 512], f32, tag="c512", bufs=3,
                                name=f"cps_{L}_{b}_{i}")
                        for i in range(R // 2)
                    ]
                    for t in range(9):
                        dy, dx = t // 3 - 1, t % 3 - 1
                        for half in (0, 1):
                            for cp in range(R // 2):
                                rr = r0 + 2 * cp
                                off = yoff(loc(rr, dy)) + dx
                                if half == 0:
                                    rhs = _ap(src_t[0:64, :], off, [[PITCH, 2], [1, W]])
                                    nc.tensor.matmul(
                                        pts[cp][0:64, :], wt[0:64, t, 0:64], rhs,
                                        start=(t == 0), stop=(t == 8),
                                        tile_position=(0, 0),
                                    )
                                else:
                                    rhs = _ap(src_t[64:128, :], off, [[PITCH, 2], [1, W]])
                                    nc.tensor.matmul(
                                        pts[cp][64:128, :], wt[64:128, t, 64:128], rhs,
                                        start=(t == 0), stop=(t == 8),
                                        tile_position=(64, 64),
                                    )
                    for cp in range(R // 2):
                        rr = r0 + 2 * cp
                        dst = _ap(ysb[:], yoff(rr + slot0), [[PITCH, 2], [1, W]])
                        nc.scalar.copy(out=dst, in_=pts[cp][:])

                def do_transpose(b):
                    r0 = b * R
                    pts2 = pst.tile([128, 1024], f16, tag="tp", name=f"tp_{L}_{b}")
                    for j in range(2 * R):
                        rr = r0 + j // 2
                        cs = j % 2
                        src = _ap(ysb[:], yoff(rr + slot0) + cs * 128, [[1, 128]])
                        nc.tensor.transpose(
                            pts2[:, j * 128 : (j + 1) * 128], src, id128sb[:]
                        )
                    sp = stripp.tile([128, 2 * R, 256], f16, tag="strip",
                                     name=f"sp_{L}_{b}")
                    nc.scalar.copy(
                        out=_ap(sp[:], 0, [[256, 2 * R], [1, 128]]),
                        in_=pts2[:],
                    )
                    nc.vector.tensor_tensor(
                        _ap(sp[:], 128, [[256, 2 * R], [1, 128]]),
                        _ap(sp[:], 0, [[256, 2 * R], [1, 128]]),
                        pts2[:],
                        ALU.mult,
                    )
                    pend[b] = sp

                def do_stats(b):
                    sp = pend.pop(b)
                    for j in range(2 * R):
                        ci = b * 2 * R + j
                        nc.tensor.matmul(
                            stats[:],
                            _ap(maskpm[:], ci * 18, [[1, 18]]),
                            sp[:, j, :],
                            start=(ci == 0), stop=(ci == NST - 1),
                        )

                for b in range(NB):
                    if L == 1 and b >= 1:
                        # weave the previous layer's normalize trail between
                        # this layer's conv blocks: keep ~2 chunks ahead of
                        # what block b's taps read (rows <= 4b+4)
                        for g in (2 * b + 3, 2 * b + 4):
                            if g <= NG - 2:
                                norm_emitters[0](g)
                    do_conv(b)
                    if b >= 1:
                        do_transpose(b - 1)
                    if b >= 2:
                        do_stats(b - 2)
                do_transpose(NB - 1)
                do_stats(NB - 2)
                do_stats(NB - 1)

                # ================= stats finalize =================
                ssb = smallp.tile([18, 256], f32, tag="ssb")
                nc.scalar.copy(out=ssb[:], in_=stats[:])
                fold = ps.tile([128, 512], f32, tag="exp", bufs=3,
                               name=f"fold{L}")
                nc.tensor.matmul(fold[0:9, 0:128], id18sb[:, 0:9],
                                 _ap(ssb[:], 0, [[128, 2], [1, 64]]),
                                 start=True, stop=False, skip_group_check=True)
                nc.tensor.matmul(fold[0:9, 0:128], id18sb[:, 9:18],
                                 _ap(ssb[:], 64, [[128, 2], [1, 64]]),
                                 start=False, stop=True, skip_group_check=True)
                s12 = smallp.tile([9, 128], f32, tag="s12")
                nc.scalar.copy(out=s12[:], in_=fold[0:9, 0:128])
                mean = smallp.tile([9, 64], f32, tag="mean")
                nc.vector.tensor_scalar_mul(out=mean[:], in0=s12[:, 0:64],
                                            scalar1=rcsb[:])
                e2 = smallp.tile([9, 64], f32, tag="e2")
                nc.vector.tensor_scalar_mul(out=e2[:], in0=s12[:, 64:128],
                                            scalar1=rcsb[:])
                var = smallp.tile([9, 64], f32, tag="var")
                nc.vector.tensor_tensor(var[:], mean[:], mean[:], ALU.mult)
                nc.vector.tensor_tensor(var[:], e2[:], var[:], ALU.subtract)
                sd = smallp.tile([9, 64], f32, tag="sd")
                nc.scalar.activation(
                    out=sd[:], in_=var[:], func=mybir.ActivationFunctionType.Sqrt,
                    bias=epsap[:], scale=1.0,
                )
                rstd = smallp.tile([9, 64], f32, tag="rstd")
                nc.vector.reciprocal(out=rstd[:], in_=sd[:])
                af32 = smallp.tile([9, 64], f32, tag="af32")
                nc.vector.tensor_tensor(af32[:], rstd[:], gam[L][:], ALU.mult)
                nc.vector.tensor_copy(af[0:8, :], af32[0:8, :])
                mA = smallp.tile([9, 64], f32, tag="mA")
                nc.vector.tensor_tensor(mA[:], mean[:], af32[:], ALU.mult)
                bf32 = smallp.tile([9, 64], f32, tag="bf32")
                nc.vector.tensor_tensor(bf32[:], bet[L][:], mA[:], ALU.subtract)
                nc.vector.tensor_copy(cf[0:8, :], bf32[0:8, :])

                # ================= normalize =================
                def load_ms(mg, sfx):
                    msa = msp.tile([9, MCH], f16, tag="msa", name=f"msa{L}_{sfx}")
                    msb = msp.tile([9, MCH], f16, tag="msb", name=f"msb{L}_{sfx}")
                    nc.sync.dma_start(
                        out=msa[:],
                        in_=bass.AP(tensor=ms2ad[:].tensor, offset=mg * MCH,
                                    ap=[[HW2, 9], [1, MCH]]),
                    )
                    nc.sync.dma_start(
                        out=msb[:],
                        in_=bass.AP(tensor=ms2bd[:].tensor, offset=mg * MCH,
                                    ap=[[HW2, 9], [1, MCH]]),
                    )
                    return msa, msb

                # L0 emits the LAST chunk first so both inter-layer halo rows
                # exist early; then a few leading chunks pre-seed, and the
                # remaining chunks are woven between the next layer's conv
                # blocks (see the b-loop above).
                if L == 0:
                    gmap = {NMG - 1: load_ms(NMG - 1, "p7"),
                            0: load_ms(0, "p0"), 1: load_ms(1, "p1")}
                else:
                    gmap = {0: load_ms(0, "p0"), 1: load_ms(1, "p1")}
                stg_box = [None]

                def emit_norm(g, L=L, slot0=slot0, af=af, cf=cf, gmap=gmap,
                              load_ms=load_ms, stg_box=stg_box):
                    if g % 8 == 6:
                        mg2 = g // 8 + 2
                        if (L == 0 and g <= NG - 18) or (L == 1 and mg2 < NMG):
                            gmap[mg2] = load_ms(mg2, f"l{g}")
                    mg = g // 8
                    msa, msb = gmap[mg]
                    j = g % 8
                    base = yoff(2 * g + slot0)
                    sE = ps.tile([128, 512], f32, tag="exp", bufs=3,
                                 name=f"se{L}_{g}")
                    oE = ps.tile([128, 512], f32, tag="exp", bufs=3,
                                 name=f"oe{L}_{g}")
                    winA = msa[:, j * 512 : (j + 1) * 512]
                    winB = msb[:, j * 512 : (j + 1) * 512]
                    yv = _ap(ysb[:], base, [[PITCH, 2], [1, W]])
                    nc.tensor.matmul(sE[0:64, :], af[:], winA,
                                     start=True, stop=True, tile_position=(0, 0))
                    nc.tensor.matmul(sE[64:128, :], af[:], winB,
                                     start=True, stop=True, tile_position=(0, 64))
                    nc.tensor.matmul(oE[0:64, :], cf[:], winA,
                                     start=True, stop=True, tile_position=(0, 0))
                    nc.tensor.matmul(oE[64:128, :], cf[:], winB,
                                     start=True, stop=True, tile_position=(0, 64))
                    sEc = nrm.tile([128, 512], f16, tag="sEc", name=f"sEc{L}_{g}")
                    oEc = nrm.tile([128, 512], f16, tag="oEc", name=f"oEc{L}_{g}")
                    nc.scalar.copy(out=sEc[:], in_=sE[:])
                    nc.scalar.copy(out=oEc[:], in_=oE[:])
                    t1 = nrm.tile([128, 512], f16, tag="t1", name=f"t1{L}_{g}")
                    t2 = nrm.tile([128, 512], f16, tag="t2", name=f"t2{L}_{g}")
                    nc.vector.tensor_tensor(t1[:], yv, sEc[:], ALU.mult)
                    nc.vector.tensor_tensor(t2[:], t1[:], oEc[:], ALU.add)
                    if L == 0:
                        dst_relu = yv
                    else:
                        if g % 4 == 0:
                            stg_box[0] = stgp.tile([128, 2048], f16, tag="stg",
                                                   name=f"stg{g // 4}")
                        stg = stg_box[0]
                        dst_relu = stg[:, (g % 4) * 512 : (g % 4 + 1) * 512]
                    nc.vector.tensor_scalar_max(out=dst_relu, in0=t2[:],
                                                scalar1=0.0)
                    if L == 0:
                        if g == 0:
                            # halo: A slot HH+1 <- B row 0 (normalized)
                            nc.sync.dma_start(
                                out=_ap(ysb[0:64, :], yoff(HH + 1), [[1, W]]),
                                in_=_ap(ysb[64:128, :], yoff(1), [[1, W]]),
                            )
                        if g == NG - 1:
                            # halo: B slot 0 <- A row HH-1 (normalized)
                            nc.sync.dma_start(
                                out=_ap(ysb[64:128, :], yoff(0), [[1, W]]),
                                in_=_ap(ysb[0:64, :], yoff(HH), [[1, W]]),
                            )
                    else:
                        if g % 4 == 3:
                            grp = g // 4
                            nc.sync.dma_start(
                                out=bass.AP(
                                    tensor=out[:].tensor,
                                    offset=grp * 8 * W,
                                    ap=[[H * W, 64], [1, 2048]],
                                ),
                                in_=stg[0:64, :],
                            )
                            nc.sync.dma_start(
                                out=bass.AP(
                                    tensor=out[:].tensor,
                                    offset=HH * W + grp * 8 * W,
                                    ap=[[H * W, 64], [1, 2048]],
                                ),
                                in_=stg[64:128, :],
                            )

                norm_emitters[L] = emit_norm
                if L == 0:
                    # last chunk first (frees the B halo), then pre-seed the
                    # first few rows; the rest is woven into layer 1's conv
                    for g in (NG - 1, 0, 1, 2, 3, 4):
                        emit_norm(g)
                else:
                    for g in range(NG):
                        emit_norm(g)

    return nc


MAXW = 1


def _split_multi_waits(nc):
    """The installed walrus rejects instructions with >MAXW sync waits; hoist
    excess waits onto preceding same-engine nops."""
    nsplit = 0
    for fn in nc.m.functions:
        for blk in fn.blocks:
            insts = list(blk.instructions)
            out = []
            for inst in insts:
                si = inst.sync_info
                waits = list(si.on_wait) if (si and si.on_wait) else []
                if len(waits) > MAXW:
                    for i in range(0, len(waits) - MAXW, MAXW):
                        nop = mybir.InstNoOp(
                            name=f"WSPLIT-{nsplit}", ins=[], outs=[]
                        )
                        nsplit += 1
                        nop.engine = inst.engine
                        nop.sync_info = mybir.SyncInfo(
                            on_wait=waits[i : i + MAXW], on_update=[]
                        )
                        out.append(nop)
                    si.on_wait = waits[len(waits) - MAXW :]
                out.append(inst)
            if len(out) != len(insts):
                while len(blk.instructions):
                    blk.instructions.pop()
                for inst in out:
                    blk.instructions.append(inst)
    return nsplit


def build_nc(H=256, split_waits=True):
    _install_tile_patch()
    nc = bass.Bass()
    emit(nc, H)
    if split_waits:
        n = _split_multi_waits(nc)
        if n:
            print(f"kernel: split {n} multi-wait instructions")
    return nc


# ---------------------------------------------------------------------------
# host-side input prep
# ---------------------------------------------------------------------------
def prep_core_inputs(x_img, ids_img, w0, g0v, b0v, w1, g1v, b1v, H=256):
    """x_img [C,H,W] f32, ids_img [H,W] int -> input map for one core."""
    HH = H // 2
    NST = HH * 2
    seg = np.where(ids_img < 0, 8, ids_img).astype(np.int64)

    m = {}
    m["xh"] = np.ascontiguousarray(x_img.reshape(C, H * W).astype(np.float16))
    cnt = np.bincount(seg.reshape(-1), minlength=9)[:9]
    m["rcnt"] = (1.0 / np.maximum(cnt, 1)).astype(np.float32)

    # one-hot masks, host-precomputed
    # maskd[p, (2*rr+cs)*18 + 9*h + s] = (ids[h*HH+rr, cs*128+p] == vals[s])
    idh = ids_img.reshape(2, HH, 2, 128)              # [h, rr, cs, p]
    vals = np.arange(9)
    vals[8] = -1
    mk = (idh[..., None] == vals).astype(np.float16)  # [h, rr, cs, p, s]
    mk = mk.transpose(3, 1, 2, 0, 4)                  # [p, rr, cs, h, s]
    m["maskd"] = np.ascontiguousarray(mk.reshape(128, NST * 18))
    # segment-major masks per half: ms2{a,b}[s, px]
    flat = ids_img.reshape(2, HH * W)
    msk2 = (flat[:, None, :] == vals[None, :, None]).astype(np.float16)  # [2,9,px]
    m["ms2ad"] = np.ascontiguousarray(msk2[0])
    m["ms2bd"] = np.ascontiguousarray(msk2[1])

    for name, wmat in (("w0d", w0), ("w1d", w1)):
        wd = np.zeros((9, 128, 128), np.float16)
        for t in range(9):
            dy, dx = t // 3, t % 3
            lhsT = wmat[:, :, dy, dx].T.astype(np.float16)  # [cin, cout]
            wd[t, 0:64, 0:64] = lhsT
            wd[t, 64:128, 64:128] = lhsT
        m[name] = np.ascontiguousarray(wd.transpose(1, 0, 2))  # [ci, t, co]

    m["id128"] = np.eye(128, dtype=np.float16)
    m["id18f"] = np.eye(18, dtype=np.float32)
    m["g0"] = np.asarray(g0v, np.float32)
    m["b0"] = np.asarray(b0v, np.float32)
    m["g1"] = np.asarray(g1v, np.float32)
    m["b1"] = np.asarray(b1v, np.float32)
    return m


LAST_RESULT = None


def kernel(features, ins_indices_batch, w0, g0, b0, w1, g1, b1):
    global LAST_RESULT
    _install_ntff_shim()
    from concourse.bass_utils import run_bass_kernel_spmd
    from concourse import bass2jax as _b2j
    import traceback as _tb

    _b2j.install_neuronx_cc_hook()
    import libneuronxla as _lnx

    if not getattr(_lnx, "_ant_dbg_wrapped", False):
        _orig = _lnx.neuronx_cc

        def _dbg(*a, **k):
            try:
                return _orig(*a, **k)
            except BaseException:
                _tb.print_exc()
                raise

        _lnx.neuronx_cc = _dbg
        _lnx._ant_dbg_wrapped = True

    x = np.asarray(features, np.float32)
    ids = np.asarray(ins_indices_batch).astype(np.int64)
    w0 = np.asarray(w0, np.float32)
    w1 = np.asarray(w1, np.float32)
    N = x.shape[0]
    H = x.shape[2]

    nc = build_nc(H)
    in_maps = [
        prep_core_inputs(x[i], ids[i], w0, g0, b0, w1, g1, b1, H) for i in range(N)
    ]
    trace = bool(int(os.environ.get("BASS_KERNEL_TRACE", "0")))
    res = run_bass_kernel_spmd(nc, in_maps, list(range(N)), trace=trace)
    LAST_RESULT = res
    outs = [
        res.results[i]["out"].astype(np.float32).reshape(C, H, W) for i in range(N)
    ]
    return np.stack(outs, 0)


# revision 36
# speedup vs baseline: 1.0082x; 1.0019x over previous
"""Trainium2 Bass kernel for nn_DensePoseV1ConvXGNInsHead:
2x (conv3x3 64->64 -> per-instance BN -> ReLU) on [8,64,256,256],
data-parallel one image per NeuronCore across 8 cores.

Self-contained: only imports the system concourse stack from /opt/trn_rl_repo.
"""
import os
import sys
import types

sys.path.insert(0, "/opt/trn_rl_repo")

import numpy as np

import concourse.bass as bass
import concourse.tile as tile
from concourse import mybir
from concourse.vector_clock import ScopedClock

f16 = mybir.dt.float16
f32 = mybir.dt.float32
ALU = mybir.AluOpType

C = 64          # channels
W = 256         # image width
PITCH = 272     # padded row pitch (16 left pad + 256 data; borrows next row's pad)
LP = 16         # left pad elements
R = 4           # conv rows per block (per half)
EPS = 1e-5

# ---------------------------------------------------------------------------
# walrus workaround: split the Tile exit-drain's sem waits (installed walrus
# rejects instructions with >2 sync waits)
# ---------------------------------------------------------------------------
_patched = False


def _install_tile_patch():
    global _patched
    if _patched:
        return
    _patched = True

    def _drain_and_barrier(self, tick_clock, wait_clock):
        nc = self.nc
        drain_inst = nc.sync.drain()
        wait_clock.add_sem_waits(
            drain_inst.ins, ScopedClock({None: tick_clock.global_clock})
        )
        si = drain_inst.ins.sync_info
        waits = list(si.on_wait or [])
        if len(waits) > 1:
            si.on_wait = waits[:1]
            for i in range(1, len(waits)):
                nop = nc.sync.nop()
                nop.ins.sync_info = mybir.SyncInfo(
                    on_wait=waits[i : i + 1], on_update=[]
                )
        nc.all_engine_barrier()
        popped = nc._tile_sem_poison_stack.pop()
        assert popped is self._sem_poison
        nc.clear_and_free_semaphores(list(self.sems.allocated().values()))
        nc.all_engine_barrier()

    tile.TileContext._drain_and_barrier = _drain_and_barrier


# ---------------------------------------------------------------------------
# NTFF profiling shim (antenv.axon_hooks is absent in this image)
# ---------------------------------------------------------------------------
def _install_ntff_shim():
    if "antenv.axon_hooks" in sys.modules:
        return
    mod = types.ModuleType("antenv.axon_hooks")
    state = {"hook": None}
    mod.set_axon_ntff_profile_hook = lambda h: state.__setitem__("hook", h)
    mod.get_axon_ntff_profile_hook = lambda: state["hook"]
    sys.modules["antenv.axon_hooks"] = mod
    try:
        import antenv

        antenv.axon_hooks = mod
    except ImportError:
        pass
    try:
        from trn_agent_boot.trn_boot import _ntff_profile_via_ctypes

        h = _ntff_profile_via_ctypes("/opt/axon/libaxon_pjrt.so")
        mod.set_axon_ntff_profile_hook(h)
    except Exception:
        pass


def yoff(slot):
    return slot * PITCH + LP


def _ap(base_ap, offset_elems, dims):
    """Build a sub-AP of base_ap at +offset (elements), with given free dims."""
    return bass.AP(
        tensor=base_ap.tensor,
        offset=base_ap.offset + offset_elems,
        ap=[base_ap.ap[0]] + dims,
    )


def emit(nc, H):
    """Emit the full 2-layer kernel for an HxW image (H=256 in production)."""
    HH = H // 2
    NB = HH // R            # conv blocks per layer
    NST = HH * 2            # stats chunks (128 px each) per layer
    NG = HH // 2            # normalize chunks (2 rows x both halves) per layer
    HW2 = HH * W
    assert HH % R == 0

    xh = nc.declare_dram_parameter("xh", [C, H * W], f16, isOutput=False)
    maskd = nc.declare_dram_parameter("maskd", [128, NST * 18], f16, isOutput=False)
    ms2ad = nc.declare_dram_parameter("ms2ad", [9, HW2], f16, isOutput=False)
    ms2bd = nc.declare_dram_parameter("ms2bd", [9, HW2], f16, isOutput=False)
    rcnt = nc.declare_dram_parameter("rcnt", [9], f32, isOutput=False)
    w0d = nc.declare_dram_parameter("w0d", [128, 9, 128], f16, isOutput=False)
    w1d = nc.declare_dram_parameter("w1d", [128, 9, 128], f16, isOutput=False)
    id128 = nc.declare_dram_parameter("id128", [128, 128], f16, isOutput=False)
    id18f = nc.declare_dram_parameter("id18f", [18, 18], f32, isOutput=False)
    g0 = nc.declare_dram_parameter("g0", [C], f32, isOutput=False)
    b0 = nc.declare_dram_parameter("b0", [C], f32, isOutput=False)
    g1 = nc.declare_dram_parameter("g1", [C], f32, isOutput=False)
    b1 = nc.declare_dram_parameter("b1", [C], f32, isOutput=False)
    out = nc.declare_dram_parameter("out", [C, H * W], f16, isOutput=True)

    with tile.TileContext(nc) as tc:
        import contextlib

        with contextlib.ExitStack() as ctx:
            const = ctx.enter_context(tc.tile_pool(name="const", bufs=1))
            stripp = ctx.enter_context(tc.tile_pool(name="stripp", bufs=3))
            msp = ctx.enter_context(tc.tile_pool(name="msp", bufs=3))
            nrm = ctx.enter_context(tc.tile_pool(name="nrm", bufs=6))
            stgp = ctx.enter_context(tc.tile_pool(name="stgp", bufs=3))
            smallp = ctx.enter_context(tc.tile_pool(name="smallp", bufs=2))
            ps = ctx.enter_context(tc.tile_pool(name="ps", bufs=4, space="PSUM"))
            pst = ctx.enter_context(tc.tile_pool(name="pst", bufs=1, space="PSUM"))
            pss = ctx.enter_context(tc.tile_pool(name="pss", bufs=1, space="PSUM"))

            # ---- layer-0 weights first on the sync queue so conv starts ASAP
            # (layer-1 weights are loaded at the top of layer 1)
            wts = []
            for wd in (w0d, w1d):
                wt = const.tile([128, 9, 128], f16, tag="wt")
                wts.append(wt)
            nc.sync.dma_start(out=wts[0][:, 0:3, :], in_=w0d[:, 0:3, :])
            nc.sync.dma_start(out=wts[0][:, 3:9, :], in_=w0d[:, 3:9, :])
            wz = const.tile([128, 64], f16)
            nc.vector.memset(wz[:], 0.0)
            pwarm = ps.tile([128, 512], f32, tag="c512", bufs=3, name="pwarm")
            for _ in range(45):
                nc.tensor.matmul(pwarm[0:64, 0:64], wz[0:64, :], wz[0:64, :],
                                 start=True, stop=True, tile_position=(0, 0))

            # ---- persistent y buffer + x staging (pads zeroed once)
            ysb = const.tile([128, (HH + 2) * PITCH + LP], f16)
            xb0 = const.tile([128, (R + 2) * PITCH + LP], f16, tag="xb0")
            xb1 = const.tile([128, (R + 2) * PITCH + LP], f16, tag="xb1")
            nc.vector.memset(xb0[:], 0.0)
            nc.vector.memset(xb1[:], 0.0)
            xbs = [xb0, xb1]
            # ysb: zero the pad strips + the two halo slots (0 and HH+1)
            nc.vector.memset(_ap(ysb[:], 0, [[PITCH, HH + 2], [1, LP]]), 0.0)
            nc.vector.memset(_ap(ysb[:], (HH + 2) * PITCH, [[1, LP]]), 0.0)
            nc.vector.memset(_ap(ysb[:], yoff(0), [[1, W]]), 0.0)
            nc.vector.memset(_ap(ysb[:], yoff(HH + 1), [[1, W]]), 0.0)

            # ---- constants on the scalar hwdge queue (off the critical path)
            id128sb = const.tile([128, 128], f16)
            nc.scalar.dma_start(out=id128sb[:], in_=id128[:])
            id18sb = const.tile([18, 18], f32)
            nc.scalar.dma_start(out=id18sb[:], in_=id18f[:])
            maskpm = const.tile([128, NST * 18], f16)
            nc.scalar.dma_start(out=maskpm[:], in_=maskd[:])
            rcsb = const.tile([9, 1], f32)
            nc.scalar.dma_start(out=rcsb[:], in_=rcnt[:].rearrange("(a b) -> a b", b=1))
            epsap = const.tile([9, 1], f32)
            nc.vector.memset(epsap[:], EPS)
            one16 = const.tile([1, 64], f16)
            zero16 = const.tile([1, 64], f16)
            nc.vector.memset(one16[:], 1.0)
            nc.vector.memset(zero16[:], 0.0)
            gam = []
            bet = []
            for gg, bb in ((g0, b0), (g1, b1)):
                gt = const.tile([9, 64], f32, tag="gam")
                bt = const.tile([9, 64], f32, tag="bet")
                nc.scalar.dma_start(out=gt[:], in_=gg[:].partition_broadcast(9))
                nc.scalar.dma_start(out=bt[:], in_=bb[:].partition_broadcast(9))
                gam.append(gt)
                bet.append(bt)

            MCH = 4096          # ms2 stream chunk (8 normalize windows)
            NMG = HW2 // MCH    # ms2 groups per layer
            norm_emitters = [None, None]

            for L in (0, 1):
                wt = wts[L]
                if L == 1:
                    nc.scalar.dma_start(out=wts[1][:], in_=w1d[:])
                slot0 = 1 if L == 0 else 0   # y row r lives at slot r+slot0
                stats = pss.tile([18, 256], f32, tag="stats", name=f"stats{L}")
                # af/bf allocated early: background row (8) is constant and is
                # filled off the critical path; rows 0:8 come from the finalize
                af = smallp.tile([9, 64], f16, tag="af", name=f"af{L}")
                cf = smallp.tile([9, 64], f16, tag="cf", name=f"cf{L}")
                nc.scalar.dma_start(out=af[8:9, :], in_=one16[:])
                nc.scalar.dma_start(out=cf[8:9, :], in_=zero16[:])

                # ================= conv + stats (depth-2 pipeline) ==========
                pend = {}     # b -> psum chunks awaiting transpose/stats

                def do_conv(b):
                    r0 = b * R
                    if L == 0:
                        xb = xbs[b % 2]
                        if b == NB - 1:
                            # bottom halo of B half must be zero (slot R+1)
                            nc.vector.memset(
                                xb[64:128, (R + 1) * PITCH + LP : (R + 1) * PITCH + LP + W],
                                0.0,
                            )
                        lo_a = r0 - 1
                        s_a = 0
                        if b == 0:
                            lo_a, s_a = 0, 1
                        n_a = r0 + R - lo_a + 1
                        nc.sync.dma_start(
                            out=_ap(xb[0:64, :], yoff(s_a), [[PITCH, n_a], [1, W]]),
                            in_=bass.AP(
                                tensor=xh[:].tensor,
                                offset=lo_a * W,
                                ap=[[H * W, 64], [W, n_a], [1, W]],
                            ),
                        )
                        hb_lo = HH + r0 - 1
                        n_b = R + 2 if b < NB - 1 else R + 1
                        nc.sync.dma_start(
                            out=_ap(xb[64:128, :], yoff(0), [[PITCH, n_b], [1, W]]),
                            in_=bass.AP(
                                tensor=xh[:].tensor,
                                offset=hb_lo * W,
                                ap=[[H * W, 64], [W, n_b], [1, W]],
                            ),
                        )
                        src_t = xb
                        loc = lambda rr, dy: (rr - r0 + 1 + dy)  # slot in xb
                    else:
                        src_t = ysb
                        loc = lambda rr, dy: (rr + dy + 1)       # y1 slot

                    pts = [
                        ps.tile([128, 512], f32, tag="c512", bufs=3,
                                name=f"cps_{L}_{b}_{i}")
                        for i in range(R // 2)
                    ]
                    for t in range(9):
                        dy, dx = t // 3 - 1, t % 3 - 1
                        for half in (0, 1):
                            for cp in range(R // 2):
                                rr = r0 + 2 * cp
                                off = yoff(loc(rr, dy)) + dx
                                if half == 0:
                                    rhs = _ap(src_t[0:64, :], off, [[PITCH, 2], [1, W]])
                                    nc.tensor.matmul(
                                        pts[cp][0:64, :], wt[0:64, t, 0:64], rhs,
                                        start=(t == 0), stop=(t == 8),
                                        tile_position=(0, 0),
                                    )
                                else:
                                    rhs = _ap(src_t[64:128, :], off, [[PITCH, 2], [1, W]])
                                    nc.tensor.matmul(
                                        pts[cp][64:128, :], wt[64:128, t, 64:128], rhs,
                                        start=(t == 0), stop=(t == 8),
                                        tile_position=(64, 64),
                                    )
                    for cp in range(R // 2):
                        rr = r0 + 2 * cp
                        dst = _ap(ysb[:], yoff(rr + slot0), [[PITCH, 2], [1, W]])
                        nc.scalar.copy(out=dst, in_=pts[cp][:])

                def do_transpose(b):
                    r0 = b * R
                    pts2 = pst.tile([128, 1024], f16, tag="tp", name=f"tp_{L}_{b}")
                    for j in range(2 * R):
                        rr = r0 + j // 2
                        cs = j % 2
                        src = _ap(ysb[:], yoff(rr + slot0) + cs * 128, [[1, 128]])
                        nc.tensor.transpose(
                            pts2[:, j * 128 : (j + 1) * 128], src, id128sb[:]
                        )
                    sp = stripp.tile([128, 2 * R, 256], f16, tag="strip",
                                     name=f"sp_{L}_{b}")
                    nc.scalar.copy(
                        out=_ap(sp[:], 0, [[256, 2 * R], [1, 128]]),
                        in_=pts2[:],
                    )
                    nc.vector.tensor_tensor(
                        _ap(sp[:], 128, [[256, 2 * R], [1, 128]]),
                        _ap(sp[:], 0, [[256, 2 * R], [1, 128]]),
                        pts2[:],
                        ALU.mult,
                    )
                    pend[b] = sp

                def do_stats(b):
                    sp = pend.pop(b)
                    for j in range(2 * R):
                        ci = b * 2 * R + j
                        nc.tensor.matmul(
                            stats[:],
                            _ap(maskpm[:], ci * 18, [[1, 18]]),
                            sp[:, j, :],
                            start=(ci == 0), stop=(ci == NST - 1),
                        )

                for b in range(NB):
                    if L == 1 and b >= 1:
                        # weave the previous layer's normalize trail between
                        # this layer's conv blocks: keep ~2 chunks ahead of
                        # what block b's taps read (rows <= 4b+4)
                        for g in (2 * b + 5, 2 * b + 6):
                            if g <= NG - 2:
                                norm_emitters[0](g)
                    do_conv(b)
                    if b >= 1:
                        do_transpose(b - 1)
                    if b >= 2:
                        do_stats(b - 2)
                do_transpose(NB - 1)
                do_stats(NB - 2)
                do_stats(NB - 1)

                # ================= stats finalize =================
                ssb = smallp.tile([18, 256], f32, tag="ssb")
                nc.scalar.copy(out=ssb[:], in_=stats[:])
                fold = ps.tile([128, 512], f32, tag="exp", bufs=3,
                               name=f"fold{L}")
                nc.tensor.matmul(fold[0:9, 0:128], id18sb[:, 0:9],
                                 _ap(ssb[:], 0, [[128, 2], [1, 64]]),
                                 start=True, stop=False, skip_group_check=True)
                nc.tensor.matmul(fold[0:9, 0:128], id18sb[:, 9:18],
                                 _ap(ssb[:], 64, [[128, 2], [1, 64]]),
                                 start=False, stop=True, skip_group_check=True)
                s12 = smallp.tile([9, 128], f32, tag="s12")
                nc.scalar.copy(out=s12[:], in_=fold[0:9, 0:128])
                mean = smallp.tile([9, 64], f32, tag="mean")
                nc.vector.tensor_scalar_mul(out=mean[:], in0=s12[:, 0:64],
                                            scalar1=rcsb[:])
                e2 = smallp.tile([9, 64], f32, tag="e2")
                nc.vector.tensor_scalar_mul(out=e2[:], in0=s12[:, 64:128],
                                            scalar1=rcsb[:])
                var = smallp.tile([9, 64], f32, tag="var")
                nc.vector.tensor_tensor(var[:], mean[:], mean[:], ALU.mult)
                nc.vector.tensor_tensor(var[:], e2[:], var[:], ALU.subtract)
                sd = smallp.tile([9, 64], f32, tag="sd")
                nc.scalar.activation(
                    out=sd[:], in_=var[:], func=mybir.ActivationFunctionType.Sqrt,
                    bias=epsap[:], scale=1.0,
                )
                rstd = smallp.tile([9, 64], f32, tag="rstd")
                nc.vector.reciprocal(out=rstd[:], in_=sd[:])
                af32 = smallp.tile([9, 64], f32, tag="af32")
                nc.vector.tensor_tensor(af32[:], rstd[:], gam[L][:], ALU.mult)
                nc.vector.tensor_copy(af[0:8, :], af32[0:8, :])
                mA = smallp.tile([9, 64], f32, tag="mA")
                nc.vector.tensor_tensor(mA[:], mean[:], af32[:], ALU.mult)
                bf32 = smallp.tile([9, 64], f32, tag="bf32")
                nc.vector.tensor_tensor(bf32[:], bet[L][:], mA[:], ALU.subtract)
                nc.vector.tensor_copy(cf[0:8, :], bf32[0:8, :])

                # ================= normalize =================
                def load_ms(mg, sfx):
                    msa = msp.tile([9, MCH], f16, tag="msa", name=f"msa{L}_{sfx}")
                    msb = msp.tile([9, MCH], f16, tag="msb", name=f"msb{L}_{sfx}")
                    nc.sync.dma_start(
                        out=msa[:],
                        in_=bass.AP(tensor=ms2ad[:].tensor, offset=mg * MCH,
                                    ap=[[HW2, 9], [1, MCH]]),
                    )
                    nc.sync.dma_start(
                        out=msb[:],
                        in_=bass.AP(tensor=ms2bd[:].tensor, offset=mg * MCH,
                                    ap=[[HW2, 9], [1, MCH]]),
                    )
                    return msa, msb

                # L0 emits the LAST chunk first so both inter-layer halo rows
                # exist early; then a few leading chunks pre-seed, and the
                # remaining chunks are woven between the next layer's conv
                # blocks (see the b-loop above).
                if L == 0:
                    gmap = {NMG - 1: load_ms(NMG - 1, "p7"),
                            0: load_ms(0, "p0"), 1: load_ms(1, "p1")}
                else:
                    gmap = {0: load_ms(0, "p0"), 1: load_ms(1, "p1")}
                stg_box = [None]

                def emit_norm(g, L=L, slot0=slot0, af=af, cf=cf, gmap=gmap,
                              load_ms=load_ms, stg_box=stg_box):
                    if g % 8 == 6:
                        mg2 = g // 8 + 2
                        if (L == 0 and g <= NG - 18) or (L == 1 and mg2 < NMG):
                            gmap[mg2] = load_ms(mg2, f"l{g}")
                    mg = g // 8
                    msa, msb = gmap[mg]
                    j = g % 8
                    base = yoff(2 * g + slot0)
                    sE = ps.tile([128, 512], f32, tag="exp", bufs=3,
                                 name=f"se{L}_{g}")
                    oE = ps.tile([128, 512], f32, tag="exp", bufs=3,
                                 name=f"oe{L}_{g}")
                    winA = msa[:, j * 512 : (j + 1) * 512]
                    winB = msb[:, j * 512 : (j + 1) * 512]
                    yv = _ap(ysb[:], base, [[PITCH, 2], [1, W]])
                    nc.tensor.matmul(sE[0:64, :], af[:], winA,
                                     start=True, stop=True, tile_position=(0, 0))
                    nc.tensor.matmul(sE[64:128, :], af[:], winB,
                                     start=True, stop=True, tile_position=(0, 64))
                    nc.tensor.matmul(oE[0:64, :], cf[:], winA,
                                     start=True, stop=True, tile_position=(0, 0))
                    nc.tensor.matmul(oE[64:128, :], cf[:], winB,
                                     start=True, stop=True, tile_position=(0, 64))
                    sEc = nrm.tile([128, 512], f16, tag="sEc", name=f"sEc{L}_{g}")
                    oEc = nrm.tile([128, 512], f16, tag="oEc", name=f"oEc{L}_{g}")
                    nc.scalar.copy(out=sEc[:], in_=sE[:])
                    nc.scalar.copy(out=oEc[:], in_=oE[:])
                    t1 = nrm.tile([128, 512], f16, tag="t1", name=f"t1{L}_{g}")
                    t2 = nrm.tile([128, 512], f16, tag="t2", name=f"t2{L}_{g}")
                    nc.vector.tensor_tensor(t1[:], yv, sEc[:], ALU.mult)
                    nc.vector.tensor_tensor(t2[:], t1[:], oEc[:], ALU.add)
                    if L == 0:
                        dst_relu = yv
                    else:
                        if g % 4 == 0:
                            stg_box[0] = stgp.tile([128, 2048], f16, tag="stg",
                                                   name=f"stg{g // 4}")
                        stg = stg_box[0]
                        dst_relu = stg[:, (g % 4) * 512 : (g % 4 + 1) * 512]
                    nc.vector.tensor_scalar_max(out=dst_relu, in0=t2[:],
                                                scalar1=0.0)
                    if L == 0:
                        if g == 0:
                            # halo: A slot HH+1 <- B row 0 (normalized)
                            nc.scalar.dma_start(
                                out=_ap(ysb[0:64, :], yoff(HH + 1), [[1, W]]),
                                in_=_ap(ysb[64:128, :], yoff(1), [[1, W]]),
                            )
                        if g == NG - 1:
                            # halo: B slot 0 <- A row HH-1 (normalized)
                            nc.scalar.dma_start(
                                out=_ap(ysb[64:128, :], yoff(0), [[1, W]]),
                                in_=_ap(ysb[0:64, :], yoff(HH), [[1, W]]),
                            )
                    else:
                        grp = g // 4
                        last = grp == NG // 4 - 1
                        if (not last and g % 4 == 3) or (last and g % 2 == 1):
                            co = 0 if (not last or g % 4 == 1) else 1024
                            n = 2048 if not last else 1024
                            nc.sync.dma_start(
                                out=bass.AP(
                                    tensor=out[:].tensor,
                                    offset=grp * 8 * W + co,
                                    ap=[[H * W, 64], [1, n]],
                                ),
                                in_=stg[0:64, co : co + n],
                            )
                            nc.sync.dma_start(
                                out=bass.AP(
                                    tensor=out[:].tensor,
                                    offset=HH * W + grp * 8 * W + co,
                                    ap=[[H * W, 64], [1, n]],
                                ),
                                in_=stg[64:128, co : co + n],
                            )

                norm_emitters[L] = emit_norm
                if L == 0:
                    # last chunk first (frees the B halo), then pre-seed the
                    # first few rows; the rest is woven into layer 1's conv
                    for g in (NG - 1, 0, 1, 2, 3, 4, 5, 6):
                        emit_norm(g)
                else:
                    for g in range(NG):
                        emit_norm(g)

    return nc


MAXW = 1


def _split_multi_waits(nc):
    """The installed walrus rejects instructions with >MAXW sync waits; hoist
    excess waits onto preceding same-engine nops."""
    nsplit = 0
    for fn in nc.m.functions:
        for blk in fn.blocks:
            insts = list(blk.instructions)
            out = []
            for inst in insts:
                si = inst.sync_info
                waits = list(si.on_wait) if (si and si.on_wait) else []
                if len(waits) > MAXW:
                    for i in range(0, len(waits) - MAXW, MAXW):
                        nop = mybir.InstNoOp(
                            name=f"WSPLIT-{nsplit}", ins=[], outs=[]
                        )
                        nsplit += 1
                        nop.engine = inst.engine
                        nop.sync_info = mybir.SyncInfo(
                            on_wait=waits[i : i + MAXW], on_update=[]
                        )
                        out.append(nop)
                    si.on_wait = waits[len(waits) - MAXW :]
                out.append(inst)
            if len(out) != len(insts):
                while len(blk.instructions):
                    blk.instructions.pop()
                for inst in out:
                    blk.instructions.append(inst)
    return nsplit


def build_nc(H=256, split_waits=True):
    _install_tile_patch()
    nc = bass.Bass()
    emit(nc, H)
    if split_waits:
        n = _split_multi_waits(nc)
        if n:
            print(f"kernel: split {n} multi-wait instructions")
    return nc


# ---------------------------------------------------------------------------
# host-side input prep
# ---------------------------------------------------------------------------
def prep_core_inputs(x_img, ids_img, w0, g0v, b0v, w1, g1v, b1v, H=256):
    """x_img [C,H,W] f32, ids_img [H,W] int -> input map for one core."""
    HH = H // 2
    NST = HH * 2
    seg = np.where(ids_img < 0, 8, ids_img).astype(np.int64)

    m = {}
    m["xh"] = np.ascontiguousarray(x_img.reshape(C, H * W).astype(np.float16))
    cnt = np.bincount(seg.reshape(-1), minlength=9)[:9]
    m["rcnt"] = (1.0 / np.maximum(cnt, 1)).astype(np.float32)

    # one-hot masks, host-precomputed
    # maskd[p, (2*rr+cs)*18 + 9*h + s] = (ids[h*HH+rr, cs*128+p] == vals[s])
    idh = ids_img.reshape(2, HH, 2, 128)              # [h, rr, cs, p]
    vals = np.arange(9)
    vals[8] = -1
    mk = (idh[..., None] == vals).astype(np.float16)  # [h, rr, cs, p, s]
    mk = mk.transpose(3, 1, 2, 0, 4)                  # [p, rr, cs, h, s]
    m["maskd"] = np.ascontiguousarray(mk.reshape(128, NST * 18))
    # segment-major masks per half: ms2{a,b}[s, px]
    flat = ids_img.reshape(2, HH * W)
    msk2 = (flat[:, None, :] == vals[None, :, None]).astype(np.float16)  # [2,9,px]
    m["ms2ad"] = np.ascontiguousarray(msk2[0])
    m["ms2bd"] = np.ascontiguousarray(msk2[1])

    for name, wmat in (("w0d", w0), ("w1d", w1)):
        wd = np.zeros((9, 128, 128), np.float16)
        for t in range(9):
            dy, dx = t // 3, t % 3
            lhsT = wmat[:, :, dy, dx].T.astype(np.float16)  # [cin, cout]
            wd[t, 0:64, 0:64] = lhsT
            wd[t, 64:128, 64:128] = lhsT
        m[name] = np.ascontiguousarray(wd.transpose(1, 0, 2))  # [ci, t, co]

    m["id128"] = np.eye(128, dtype=np.float16)
    m["id18f"] = np.eye(18, dtype=np.float32)
    m["g0"] = np.asarray(g0v, np.float32)
    m["b0"] = np.asarray(b0v, np.float32)
    m["g1"] = np.asarray(g1v, np.float32)
    m["b1"] = np.asarray(b1v, np.float32)
    return m


LAST_RESULT = None


def kernel(features, ins_indices_batch, w0, g0, b0, w1, g1, b1):
    global LAST_RESULT
    _install_ntff_shim()
    from concourse.bass_utils import run_bass_kernel_spmd
    from concourse import bass2jax as _b2j
    import traceback as _tb

    _b2j.install_neuronx_cc_hook()
    import libneuronxla as _lnx

    if not getattr(_lnx, "_ant_dbg_wrapped", False):
        _orig = _lnx.neuronx_cc

        def _dbg(*a, **k):
            try:
                return _orig(*a, **k)
            except BaseException:
                _tb.print_exc()
                raise

        _lnx.neuronx_cc = _dbg
        _lnx._ant_dbg_wrapped = True

    x = np.asarray(features, np.float32)
    ids = np.asarray(ins_indices_batch).astype(np.int64)
    w0 = np.asarray(w0, np.float32)
    w1 = np.asarray(w1, np.float32)
    N = x.shape[0]
    H = x.shape[2]

    nc = build_nc(H)
    in_maps = [
        prep_core_inputs(x[i], ids[i], w0, g0, b0, w1, g1, b1, H) for i in range(N)
    ]
    trace = bool(int(os.environ.get("BASS_KERNEL_TRACE", "0")))
    res = run_bass_kernel_spmd(nc, in_maps, list(range(N)), trace=trace)
    LAST_RESULT = res
    outs = [
        res.results[i]["out"].astype(np.float32).reshape(C, H, W) for i in range(N)
    ]
    return np.stack(outs, 0)
